# revision 2
# baseline (speedup 1.0000x reference)
"""Trainium2 Bass kernel for the scalar-input GRU (B=512, T=128, H=512) + ReLU/Linear head.

Data-parallel over batch across 8 NeuronCores (64 rows each); per core the 64
rows run as W=2 interleaved waves of 32 so one wave's gate algebra overlaps the
other wave's matmuls.

v2: latency-oriented rewrite.
- All W_hh @ h matmuls run as fp8e4 DoubleRow (0.5 cyc/row vs bf16's 1.0, and
  K=256 per matmul vs 128): 24 DR matmuls/wave-step replace 48 bf16 ones.
  W_hh is scaled x64 into fp8's normal range; h state is held directly in
  fp8e4 (numerically validated: final rel err ~9e-3 vs the 2e-2 gate).
- The n-gate path is fused:  m = (P_n * 1/64) * r  in ONE scalar_tensor_tensor
  op, then  m2 = P_gn + m  via affine_then_add (P_gn = gx_n from tiny PE aug
  matmuls).  No PE "gpairs" round trip.
- (1-z) and z*h run on the otherwise-idle GPSIMD engine, off the critical
  path; the post-tanh tail is u = (1-z)*n ; h' = u + z*h (2 DVE ops).
- PSUM scale bookkeeping: P_r/P_z/P_n carry x64 (weights scaled); sigmoid
  applies scale=1/64; P_gn is unscaled.
"""

import sys

sys.path.insert(0, "/opt/trn_rl_repo")

import numpy as np

import concourse.bacc as bacc
import concourse.bass as bass
import concourse.mybir as mybir
import concourse.tile as tile
from concourse.bass_utils import run_bass_kernel_spmd
from concourse.masks import make_identity

N_CORES = 8
B_FULL, T_FULL, H = 512, 128, 512
B = B_FULL // N_CORES  # 64 batch rows per core
W = 2  # waves per core
BW = B // W  # 32 rows per wave
G3 = 3 * H  # 1536
NK = H // 128  # 4 contraction chunks
NJ = 4  # j-chunks per gate (H/128)
F32 = mybir.dt.float32
BF16 = mybir.dt.bfloat16
FP8 = mybir.dt.float8e4
AF = mybir.ActivationFunctionType
ALU = mybir.AluOpType
DR = mybir.MatmulPerfMode.DoubleRow
WSC = 64.0  # fp8 weight scale


def build_nc(T: int = T_FULL) -> bass.Bass:
    nc = bacc.Bacc("TRN2", target_bir_lowering=False, debug=False)

    x_d = nc.dram_tensor("x", [B, T], F32, kind="ExternalInput")
    whh_d = nc.dram_tensor("w_hh", [G3, H], F32, kind="ExternalInput")
    wih_d = nc.dram_tensor("w_ih", [G3, 1], F32, kind="ExternalInput")
    bih_d = nc.dram_tensor("b_ih", [G3], F32, kind="ExternalInput")
    bhh_d = nc.dram_tensor("b_hh", [G3], F32, kind="ExternalInput")
    fcw_d = nc.dram_tensor("fc_w", [1, H], F32, kind="ExternalInput")
    fcb_d = nc.dram_tensor("fc_b", [1], F32, kind="ExternalInput")
    out_d = nc.dram_tensor("out", [B, 1], F32, kind="ExternalOutput")

    with tile.TileContext(nc) as tc:
        _body(tc, T, x_d, whh_d, wih_d, bih_d, bhh_d, fcw_d, fcb_d, out_d)
    nc.compile()
    return nc


def _body(tc, T, x_d, whh_d, wih_d, bih_d, bhh_d, fcw_d, fcb_d, out_d):
    nc = tc.nc
    with (
        tc.tile_pool(name="const", bufs=1) as cpool,
        tc.tile_pool(name="state", bufs=3) as spool,
        tc.tile_pool(name="work", bufs=3) as wpool,
        tc.tile_pool(name="psmain", bufs=2, space="PSUM") as ppool,
    ):
        # ---- one-time prep ----
        # w_hh staged first (it gates the transposes), in chunk DMAs so the
        # first transposes start while the rest streams in
        wstage = cpool.tile([128, (G3 // 128) * H], F32)
        for cg in range(12):
            nc.sync.dma_start(
                out=wstage[:, cg * H : (cg + 1) * H],
                in_=whh_d[cg * 128 : (cg + 1) * 128, :],
            )

        ident128 = cpool.tile([128, 128], F32)
        make_identity(nc, ident128)
        ident64 = cpool.tile([64, 64], F32)
        make_identity(nc, ident64)

        # small input loads, spread over issue queues
        x_sb = cpool.tile([B, T], F32)
        nc.scalar.dma_start(out=x_sb[:, :], in_=x_d[:, :])
        wi12 = cpool.tile([12, 128], F32)
        nc.scalar.dma_start(
            out=wi12[:, :], in_=wih_d[:, :].rearrange("(p c) one -> p (c one)", p=12)
        )
        bs12 = cpool.tile([12, 128], F32)
        nc.gpsimd.dma_start(
            out=bs12[:, :], in_=bhh_d[None, :].rearrange("one (p c) -> (one p) c", p=12)
        )
        bi12 = cpool.tile([12, 128], F32)
        nc.gpsimd.dma_start(
            out=bi12[:, :], in_=bih_d[None, :].rearrange("one (p c) -> (one p) c", p=12)
        )
        fcwf = cpool.tile([128, NK], F32)
        nc.scalar.dma_start(
            out=fcwf[:, :],
            in_=fcw_d[:, :]
            .rearrange("one (k p) -> one k p", p=128)
            .transpose([2, 0, 1])
            .rearrange("p one k -> p (one k)"),
        )
        fcbf = cpool.tile([1, 1], F32)
        nc.gpsimd.dma_start(out=fcbf[:, :], in_=fcb_d[None, :])
        onesf = cpool.tile([1, B], F32)
        nc.gpsimd.memset(onesf[:, :], 1.0)

        # bsum = b_hh + b_ih on the r/z rows (rows 0:8); row 8:12 stays b_hh_n
        nc.vector.tensor_add(bs12[0:8, :], bs12[0:8, :], bi12[0:8, :])
        # scaled copies for the x64 PSUM convention of P_r/P_z/P_n
        wi64 = cpool.tile([12, 128], F32)
        nc.vector.tensor_scalar_mul(wi64[:, :], wi12[:, :], WSC)
        bs64 = cpool.tile([12, 128], F32)
        nc.vector.tensor_scalar_mul(bs64[:, :], bs12[:, :], WSC)

        # bf16 casts of (wi64, bs64, bih, wi) into one shared tile, one DMA
        hi_all = cpool.tile([12, 512], BF16)
        nc.gpsimd.tensor_copy(hi_all[:, 0:128], wi64[:, :])
        nc.gpsimd.tensor_copy(hi_all[:, 128:256], bs64[:, :])
        nc.gpsimd.tensor_copy(hi_all[:, 256:384], bi12[:, :])
        nc.gpsimd.tensor_copy(hi_all[:, 384:512], wi12[:, :])
        scr_d = nc.dram_tensor("scr_aug", [4, 12, 128], BF16, kind="Internal")
        nc.sync.dma_start(
            out=scr_d[:, :, :].transpose([1, 0, 2]),
            in_=hi_all[:, :].rearrange("p (k c) -> p k c", k=4),
        )

        # xaug rows: (x, 1); ones via packed-uint32 memset on the idle Pool
        # engine, x row DMA-overwritten below.
        xaug = cpool.tile([2, T * B], BF16)
        nc.gpsimd.memset(xaug[:, :].bitcast(mybir.dt.uint32), 0x3F803F80)
        ones1 = cpool.tile([1, B], BF16)
        nc.gpsimd.memset(ones1[:, :], 1.0)

        # x PE-transposed into (t, b) order, staged via DRAM
        xt_ps = ppool.tile([T, B], F32, tag="psPREP", bufs=2, name="xt_ps")
        nc.tensor.transpose(xt_ps[:, :], x_sb[:, :], ident64)
        xt_b = cpool.tile([T, B], BF16)
        nc.vector.tensor_copy(xt_b[:, :], xt_ps[:, :])
        xt_scr = nc.dram_tensor("xt_scr", [T, B], BF16, kind="Internal")
        nc.scalar.dma_start(out=xt_scr[:, :], in_=xt_b[:, :])
        nc.sync.dma_start(
            out=xaug[0:1, :], in_=xt_scr[:, :].rearrange("p c -> (p c)")[None, :]
        )

        # Stationary aug tiles (PE needs partition base 0):
        #   AUG   [2, 2H]: (wi*64, bsum*64) rows for r,z; rhs = xaug (x, 1)
        #   AUGNB [1, H]:  (b_hh_n*64); rhs = ones1
        #   AUGG  [2, H]:  (wi_n, bih_n) UNSCALED for P_gn = gx_n; rhs = xaug
        AUG = cpool.tile([2, 2 * H], BF16)
        AUGNB = cpool.tile([1, H], BF16)
        AUGG = cpool.tile([2, H], BF16)

        def row_dma(q, dst, r, kind, p0, p1):
            q.dma_start(
                out=dst[r : r + 1, :],
                in_=scr_d[kind, p0:p1, :].rearrange("p c -> (p c)")[None, :],
            )

        # kinds: 0 = wi*64, 1 = bs*64 (b_hh_n*64 on rows 8:12), 2 = bih, 3 = wi
        row_dma(nc.sync, AUG, 0, 0, 0, 8)
        row_dma(nc.sync, AUG, 1, 1, 0, 8)
        row_dma(nc.scalar, AUGNB, 0, 1, 8, 12)
        row_dma(nc.scalar, AUGG, 0, 3, 8, 12)
        row_dma(nc.scalar, AUGG, 1, 2, 8, 12)

        # w_hh.T in fp8, x64, DoubleRow-packed:
        #   block bb = c*2 + half  (c = global j-chunk 0..11, half = k-half)
        #   wT8[p, bb*256 + i*128 + j] = 64 * w_hh[c*128+j, (2*half+i)*128 + p]
        # transposes rotate over 6 idle recurrence banks; evacuation copies
        # (cast fp8 + x64 scale) round-robin over DVE/ACT
        wT8 = cpool.tile([128, 24 * 256], FP8)
        prep_tags = ["psR0", "psZ0", "psN0", "psR1", "psZ1", "psN1"]
        pi = 0
        for c in range(12):
            for k in range(NK):
                tp = ppool.tile([128, 128], F32, tag=prep_tags[pi % 6], bufs=1,
                                name=f"wprep_{c}_{k}")
                nc.tensor.transpose(
                    tp[:, :], wstage[:, c * H + k * 128 : c * H + (k + 1) * 128],
                    ident128,
                )
                bb = c * 2 + k // 2
                dst = wT8[:, bb * 256 + (k % 2) * 128 : bb * 256 + (k % 2) * 128 + 128]
                if pi % 2 == 0:
                    nc.vector.tensor_scalar_mul(dst, tp[:, :], WSC)
                else:
                    nc.scalar.activation(dst, tp[:, :], AF.Copy, scale=WSC)
                pi += 1

        # state init: hT8[w] [128, NK*BW] fp8e4, col block k = h dims [128k:128k+128)
        hT8 = []
        for w in range(W):
            h0 = spool.tile([128, NK * BW], FP8, tag=f"hT{w}", name=f"hT{w}_init")
            nc.gpsimd.memset(h0[:, :], 0.0)
            hT8.append(h0)

        def drhs(w, half):
            # [128, 2, BW] moving view of the fp8 state: k-tiles (2*half, 2*half+1)
            return hT8[w][:, half * 2 * BW : (half + 1) * 2 * BW].rearrange(
                "p (i b) -> p i b", i=2
            )

        def emit_pe(w, t, ps):
            # One open accumulation group per PSUM bank at a time, so each
            # j-chunk's group is [aug, DR, DR] contiguous per bank.
            psr, psz, psn, psgn = ps
            xs = xaug[0:2, t * B + w * BW : t * B + (w + 1) * BW]
            os = ones1[0:1, w * BW : (w + 1) * BW]
            # P_gn = gx_n (unscaled); single-matmul groups, shared prep bank
            for jc in range(NJ):
                nc.tensor.matmul(
                    psgn[:, jc * BW : (jc + 1) * BW],
                    AUGG[0:2, jc * 128 : (jc + 1) * 128],
                    xs, start=True, stop=True,
                )
            # gates r (c=0..3), z (c=4..7), n (c=8..11); r first so sigmoid(r)
            # starts as early as possible
            for g, aopen in ((0, None), (2, None), (1, None)):
                pst = (psr, psz, psn)[g]
                for jc in range(NJ):
                    c = (0, 4, 8)[g] + jc
                    po = pst[:, jc * BW : (jc + 1) * BW]
                    if g < 2:
                        nc.tensor.matmul(
                            po, AUG[0:2, g * H + jc * 128 : g * H + (jc + 1) * 128],
                            xs, start=True, stop=False,
                        )
                    else:
                        nc.tensor.matmul(
                            po, AUGNB[0:1, jc * 128 : (jc + 1) * 128],
                            os, start=True, stop=False,
                        )
                    for half in range(2):
                        bb = c * 2 + half
                        nc.tensor.matmul(
                            po,
                            wT8[:, bb * 256 : (bb + 1) * 256].rearrange(
                                "p (i j) -> p i j", i=2
                            ),
                            drhs(w, half),
                            start=False, stop=(half == 1),
                            perf_mode=DR,
                        )

        def emit_sig(w, t, ps, st):
            psr, psz, psn, psgn = ps
            rz = wpool.tile([128, 8 * BW], BF16, tag=f"rz{w}", name=f"rz{w}_{t}")
            nc.scalar.activation(rz[:, 0 : 4 * BW], psr[:, :], AF.Sigmoid,
                                 scale=1.0 / WSC)
            nc.scalar.activation(rz[:, 4 * BW : 8 * BW], psz[:, :], AF.Sigmoid,
                                 scale=1.0 / WSC)
            st["rz"] = rz

        def emit_pool(w, t, st):
            # off-critical-path: cv = 1-z, q = z*h_old on GPSIMD
            rz = st["rz"]
            cv = wpool.tile([128, 4 * BW], BF16, tag=f"cv{w}", name=f"cv{w}_{t}")
            nc.gpsimd.tensor_scalar(cv[:, :], rz[:, 4 * BW : 8 * BW], 1.0, -1.0,
                                    ALU.subtract, ALU.mult)
            q = wpool.tile([128, 4 * BW], BF16, tag=f"q{w}", name=f"q{w}_{t}")
            nc.gpsimd.tensor_tensor(q[:, :], rz[:, 4 * BW : 8 * BW], hT8[w][:, :],
                                    ALU.mult)
            st["cv"], st["q"] = cv, q

        def emit_m(w, t, ps, st):
            psr, psz, psn, psgn = ps
            rz = st["rz"]
            m = wpool.tile([128, 4 * BW], BF16, tag=f"m{w}", name=f"m{w}_{t}")
            nc.vector.scalar_tensor_tensor(
                m[:, :], psn[:, :], 1.0 / WSC, rz[:, 0 : 4 * BW],
                ALU.mult, ALU.mult,
            )
            m2 = wpool.tile([128, 4 * BW], BF16, tag=f"m2{w}", name=f"m2{w}_{t}")
            nc.vector.affine_then_add(m2[:, :], psgn[:, :], m[:, :], 1.0, 0.0)
            st["m2"] = m2

        def emit_tanh(w, t, st):
            n = wpool.tile([128, 4 * BW], BF16, tag=f"n{w}", name=f"n{w}_{t}")
            nc.scalar.activation(n[:, :], st["m2"][:, :], AF.Tanh)
            st["n"] = n

        def emit_tail(w, t, st):
            n, cv, q = st["n"], st["cv"], st["q"]
            u = wpool.tile([128, 4 * BW], BF16, tag=f"u{w}", name=f"u{w}_{t}")
            nc.vector.tensor_tensor(u[:, :], cv[:, :], n[:, :], ALU.mult)
            hn = spool.tile([128, NK * BW], FP8, tag=f"hT{w}", name=f"hT{w}_{t}")
            nc.vector.tensor_tensor(hn[:, :], u[:, :], q[:, :], ALU.add)
            hT8[w] = hn

        # ---- the recurrence, fully unrolled, 2 waves interleaved ----
        sts = [{}, {}]
        pss = [None, None]
        for t in range(T):
            for w in range(W):
                psr = ppool.tile([128, 4 * BW], F32, tag=f"psR{w}", bufs=1,
                                 name=f"psr{w}_{t}")
                psz = ppool.tile([128, 4 * BW], F32, tag=f"psZ{w}", bufs=1,
                                 name=f"psz{w}_{t}")
                psn = ppool.tile([128, 4 * BW], F32, tag=f"psN{w}", bufs=1,
                                 name=f"psn{w}_{t}")
                psgn = ppool.tile([128, 4 * BW], F32, tag="psPREP", bufs=2,
                                  name=f"psgn{w}_{t}")
                pss[w] = (psr, psz, psn, psgn)
                if w == 0:
                    emit_pe(0, t, pss[0])
                    emit_sig(0, t, pss[0], sts[0])
                else:
                    emit_pe(1, t, pss[1])
                    emit_m(0, t, pss[0], sts[0])
                    emit_pool(0, t, sts[0])
                    emit_sig(1, t, pss[1], sts[1])
                    emit_tanh(0, t, sts[0])
                    emit_m(1, t, pss[1], sts[1])
                    emit_pool(1, t, sts[1])
                    emit_tail(0, t, sts[0])
                    emit_tanh(1, t, sts[1])
                    emit_tail(1, t, sts[1])

        # ---- head: out = relu(h) @ fc_w.T + fc_b ----
        pso = ppool.tile([B, 1], F32, tag="psPREP", bufs=2, name="ps_fc")
        for w in range(W):
            reluh = wpool.tile([128, NK * BW], F32, tag=f"relu{w}", name=f"relu{w}")
            nc.scalar.activation(reluh[:, :], hT8[w][:, :], AF.Relu)
            po = pso[w * BW : (w + 1) * BW, :]
            nc.tensor.matmul(
                po, onesf[:, 0:BW], fcbf[0:1, 0:1], start=True, stop=False
            )
            for k in range(NK):
                nc.tensor.matmul(
                    po,
                    reluh[:, k * BW : (k + 1) * BW],
                    fcwf[:, k : k + 1],
                    start=False, stop=(k == NK - 1),
                )
        outw = wpool.tile([B, 1], F32, tag="outw", name="out_sb")
        nc.vector.tensor_copy(outw[:, :], pso[:, :])
        nc.sync.dma_start(out=out_d[:, :], in_=outw[:, :])


_NC_CACHE: dict[int, bass.Bass] = {}


def _get_nc(T: int = T_FULL) -> bass.Bass:
    if T not in _NC_CACHE:
        _NC_CACHE[T] = build_nc(T)
    return _NC_CACHE[T]


def kernel(x, w_ih, w_hh, b_ih, b_hh, fc_w, fc_b, _trace=False, _tmpdir=None):
    x = np.ascontiguousarray(np.asarray(x, dtype=np.float32))
    nc = _get_nc(x.shape[1])
    shared = {
        "w_hh": np.ascontiguousarray(np.asarray(w_hh, np.float32)),
        "w_ih": np.ascontiguousarray(np.asarray(w_ih, np.float32)),
        "b_ih": np.ascontiguousarray(np.asarray(b_ih, np.float32)),
        "b_hh": np.ascontiguousarray(np.asarray(b_hh, np.float32)),
        "fc_w": np.ascontiguousarray(np.asarray(fc_w, np.float32)),
        "fc_b": np.ascontiguousarray(np.asarray(fc_b, np.float32)),
    }
    in_maps = [{"x": x[c * B : (c + 1) * B], **shared} for c in range(N_CORES)]
    res = run_bass_kernel_spmd(
        nc, in_maps, list(range(N_CORES)), trace=_trace, tmpdir=_tmpdir
    )
    out = np.concatenate([res.results[c]["out"] for c in range(N_CORES)], axis=0)
    if _trace:
        return out, res
    return out


# revision 7
# speedup vs baseline: 1.0498x; 1.0498x over previous
"""Trainium2 Bass kernel for the scalar-input GRU (B=512, T=128, H=512) + ReLU/Linear head.

Data-parallel over batch across 8 NeuronCores (64 rows each); per core the 64
rows run as W=2 interleaved waves of 32 so one wave's gate algebra overlaps the
other wave's matmuls.

v2: latency-oriented rewrite.
- All W_hh @ h matmuls run as fp8e4 DoubleRow (0.5 cyc/row vs bf16's 1.0, and
  K=256 per matmul vs 128): 24 DR matmuls/wave-step replace 48 bf16 ones.
  W_hh is scaled x64 into fp8's normal range; h state is held directly in
  fp8e4 (numerically validated: final rel err ~9e-3 vs the 2e-2 gate).
- The n-gate path:  m = (P_n * 1/64) * r  in ONE scalar_tensor_tensor op,
  then gx_n + m accumulates on PE (aug + identity matmul into psn2, which
  beats a DVE add: no same-engine min-delay stall) and tanh reads psn2.
- z*h runs on the otherwise-idle GPSIMD engine off the critical path;
  1-z runs on DVE (4x mode); the post-tanh tail is u = (1-z)*n ;
  h' = u + z*h (2 DVE ops).
- PSUM scale bookkeeping: P_r/P_z/P_n carry x64 (weights scaled); sigmoid
  applies scale=1/64; P_gn is unscaled.
"""

import sys

sys.path.insert(0, "/opt/trn_rl_repo")

import numpy as np

import concourse.bacc as bacc
import concourse.bass as bass
import concourse.mybir as mybir
import concourse.tile as tile
from concourse.bass_utils import run_bass_kernel_spmd
from concourse.masks import make_identity

N_CORES = 8
B_FULL, T_FULL, H = 512, 128, 512
B = B_FULL // N_CORES  # 64 batch rows per core
W = 2  # waves per core
BW = B // W  # 32 rows per wave
G3 = 3 * H  # 1536
NK = H // 128  # 4 contraction chunks
NJ = 4  # j-chunks per gate (H/128)
F32 = mybir.dt.float32
BF16 = mybir.dt.bfloat16
FP8 = mybir.dt.float8e4
AF = mybir.ActivationFunctionType
ALU = mybir.AluOpType
DR = mybir.MatmulPerfMode.DoubleRow
WSC = 64.0  # fp8 weight scale


def build_nc(T: int = T_FULL) -> bass.Bass:
    nc = bacc.Bacc("TRN2", target_bir_lowering=False, debug=False)

    x_d = nc.dram_tensor("x", [B, T], F32, kind="ExternalInput")
    whh_d = nc.dram_tensor("w_hh", [G3, H], F32, kind="ExternalInput")
    wih_d = nc.dram_tensor("w_ih", [G3, 1], F32, kind="ExternalInput")
    bih_d = nc.dram_tensor("b_ih", [G3], F32, kind="ExternalInput")
    bhh_d = nc.dram_tensor("b_hh", [G3], F32, kind="ExternalInput")
    fcw_d = nc.dram_tensor("fc_w", [1, H], F32, kind="ExternalInput")
    fcb_d = nc.dram_tensor("fc_b", [1], F32, kind="ExternalInput")
    out_d = nc.dram_tensor("out", [B, 1], F32, kind="ExternalOutput")

    with tile.TileContext(nc) as tc:
        _body(tc, T, x_d, whh_d, wih_d, bih_d, bhh_d, fcw_d, fcb_d, out_d)
    nc.compile()
    return nc


def _body(tc, T, x_d, whh_d, wih_d, bih_d, bhh_d, fcw_d, fcb_d, out_d):
    nc = tc.nc
    with (
        tc.tile_pool(name="const", bufs=1) as cpool,
        tc.tile_pool(name="state", bufs=3) as spool,
        tc.tile_pool(name="work", bufs=3) as wpool,
        tc.tile_pool(name="psmain", bufs=2, space="PSUM") as ppool,
    ):
        # ---- one-time prep ----
        # w_hh staged first (it gates the transposes), in chunk DMAs so the
        # first transposes start while the rest streams in
        wstage = cpool.tile([128, (G3 // 128) * H], F32)
        for cg in range(12):
            nc.sync.dma_start(
                out=wstage[:, cg * H : (cg + 1) * H],
                in_=whh_d[cg * 128 : (cg + 1) * 128, :],
            )

        ident128 = cpool.tile([128, 128], F32)
        make_identity(nc, ident128)
        identb = cpool.tile([128, 128], BF16)
        nc.vector.tensor_copy(identb[:, :], ident128[:, :])
        ident64 = cpool.tile([64, 64], F32)
        make_identity(nc, ident64)

        # small input loads, spread over issue queues
        x_sb = cpool.tile([B, T], F32)
        nc.scalar.dma_start(out=x_sb[:, :], in_=x_d[:, :])
        wi12 = cpool.tile([12, 128], F32)
        nc.scalar.dma_start(
            out=wi12[:, :], in_=wih_d[:, :].rearrange("(p c) one -> p (c one)", p=12)
        )
        bs12 = cpool.tile([12, 128], F32)
        nc.gpsimd.dma_start(
            out=bs12[:, :], in_=bhh_d[None, :].rearrange("one (p c) -> (one p) c", p=12)
        )
        bi12 = cpool.tile([12, 128], F32)
        nc.gpsimd.dma_start(
            out=bi12[:, :], in_=bih_d[None, :].rearrange("one (p c) -> (one p) c", p=12)
        )
        fcwf = cpool.tile([128, NK], F32)
        nc.scalar.dma_start(
            out=fcwf[:, :],
            in_=fcw_d[:, :]
            .rearrange("one (k p) -> one k p", p=128)
            .transpose([2, 0, 1])
            .rearrange("p one k -> p (one k)"),
        )
        fcbf = cpool.tile([1, 1], F32)
        nc.gpsimd.dma_start(out=fcbf[:, :], in_=fcb_d[None, :])
        onesf = cpool.tile([1, B], F32)
        nc.gpsimd.memset(onesf[:, :], 1.0)

        # bsum = b_hh + b_ih on the r/z rows (rows 0:8); row 8:12 stays b_hh_n
        nc.vector.tensor_add(bs12[0:8, :], bs12[0:8, :], bi12[0:8, :])
        # scaled copies for the x64 PSUM convention of P_r/P_z/P_n
        wi64 = cpool.tile([12, 128], F32)
        nc.vector.tensor_scalar_mul(wi64[:, :], wi12[:, :], WSC)
        bs64 = cpool.tile([12, 128], F32)
        nc.vector.tensor_scalar_mul(bs64[:, :], bs12[:, :], WSC)

        # bf16 casts of (wi64, bs64, bih, wi) into one shared tile, one DMA
        hi_all = cpool.tile([12, 512], BF16)
        nc.gpsimd.tensor_copy(hi_all[:, 0:128], wi64[:, :])
        nc.gpsimd.tensor_copy(hi_all[:, 128:256], bs64[:, :])
        nc.gpsimd.tensor_copy(hi_all[:, 256:384], bi12[:, :])
        nc.gpsimd.tensor_copy(hi_all[:, 384:512], wi12[:, :])
        scr_d = nc.dram_tensor("scr_aug", [4, 12, 128], BF16, kind="Internal")
        nc.sync.dma_start(
            out=scr_d[:, :, :].transpose([1, 0, 2]),
            in_=hi_all[:, :].rearrange("p (k c) -> p k c", k=4),
        )

        # xaug rows: (x, 1); ones via packed-uint32 memset on the idle Pool
        # engine, x row DMA-overwritten below.
        xaug = cpool.tile([2, T * B], BF16)
        nc.gpsimd.memset(xaug[:, :].bitcast(mybir.dt.uint32), 0x3F803F80)
        ones1 = cpool.tile([1, B], BF16)
        nc.gpsimd.memset(ones1[:, :], 1.0)

        # x PE-transposed into (t, b) order, staged via DRAM
        xt_ps = ppool.tile([T, B], F32, tag="psPREP", bufs=2, name="xt_ps")
        nc.tensor.transpose(xt_ps[:, :], x_sb[:, :], ident64)
        xt_b = cpool.tile([T, B], BF16)
        nc.vector.tensor_copy(xt_b[:, :], xt_ps[:, :])
        xt_scr = nc.dram_tensor("xt_scr", [T, B], BF16, kind="Internal")
        nc.scalar.dma_start(out=xt_scr[:, :], in_=xt_b[:, :])
        nc.sync.dma_start(
            out=xaug[0:1, :], in_=xt_scr[:, :].rearrange("p c -> (p c)")[None, :]
        )

        # Stationary aug tiles (PE needs partition base 0):
        #   AUG   [2, 2H]: (wi*64, bsum*64) rows for r,z; rhs = xaug (x, 1)
        #   AUGNB [1, H]:  (b_hh_n*64); rhs = ones1
        #   AUGG  [2, H]:  (wi_n, bih_n) UNSCALED for P_gn = gx_n; rhs = xaug
        AUG = cpool.tile([2, 2 * H], BF16)
        AUGNB = cpool.tile([1, H], BF16)
        AUGG = cpool.tile([2, H], BF16)

        def row_dma(q, dst, r, kind, p0, p1):
            q.dma_start(
                out=dst[r : r + 1, :],
                in_=scr_d[kind, p0:p1, :].rearrange("p c -> (p c)")[None, :],
            )

        # kinds: 0 = wi*64, 1 = bs*64 (b_hh_n*64 on rows 8:12), 2 = bih, 3 = wi
        row_dma(nc.sync, AUG, 0, 0, 0, 8)
        row_dma(nc.sync, AUG, 1, 1, 0, 8)
        row_dma(nc.scalar, AUGNB, 0, 1, 8, 12)
        row_dma(nc.scalar, AUGG, 0, 3, 8, 12)
        row_dma(nc.scalar, AUGG, 1, 2, 8, 12)

        # w_hh.T in fp8, x64, DoubleRow-packed:
        #   block bb = c*2 + half  (c = global j-chunk 0..11, half = k-half)
        #   wT8[p, bb*256 + i*128 + j] = 64 * w_hh[c*128+j, (2*half+i)*128 + p]
        # transposes rotate over 6 idle recurrence banks; evacuation copies
        # (cast fp8 + x64 scale) round-robin over DVE/ACT
        wT8 = cpool.tile([128, 24 * 256], FP8)
        prep_tags = ["psR0", "psZ0", "psN0", "psR1", "psZ1", "psN1"]
        pi = 0
        for c in range(12):
            for k in range(NK):
                tp = ppool.tile([128, 128], F32, tag=prep_tags[pi % 6], bufs=1,
                                name=f"wprep_{c}_{k}")
                nc.tensor.transpose(
                    tp[:, :], wstage[:, c * H + k * 128 : c * H + (k + 1) * 128],
                    ident128,
                )
                bb = c * 2 + k // 2
                dst = wT8[:, bb * 256 + (k % 2) * 128 : bb * 256 + (k % 2) * 128 + 128]
                if pi % 2 == 0:
                    nc.vector.tensor_scalar_mul(dst, tp[:, :], WSC)
                else:
                    nc.scalar.activation(dst, tp[:, :], AF.Copy, scale=WSC)
                pi += 1

        # state init: hT8[w] [128, NK*BW] fp8e4, col block k = h dims [128k:128k+128)
        hT8 = []
        for w in range(W):
            h0 = spool.tile([128, NK * BW], FP8, tag=f"hT{w}", name=f"hT{w}_init")
            nc.gpsimd.memset(h0[:, :], 0.0)
            hT8.append(h0)

        def drhs(w, half):
            # [128, 2, BW] moving view of the fp8 state: k-tiles (2*half, 2*half+1)
            return hT8[w][:, half * 2 * BW : (half + 1) * 2 * BW].rearrange(
                "p (i b) -> p i b", i=2
            )

        def emit_pe(w, t, ps):
            # One open accumulation group per PSUM bank at a time, so each
            # j-chunk's group is [aug, DR, DR] contiguous per bank.
            psr, psz, psn, psn2 = ps
            xs = xaug[0:2, t * B + w * BW : t * B + (w + 1) * BW]
            os = ones1[0:1, w * BW : (w + 1) * BW]
            # gates r (c=0..3), z (c=4..7), n (c=8..11); r first so sigmoid(r)
            # starts as early as possible, n second (m needs it)
            for g, aopen in ((0, None), (2, None), (1, None)):
                pst = (psr, psz, psn)[g]
                for jc in range(NJ):
                    c = (0, 4, 8)[g] + jc
                    po = pst[:, jc * BW : (jc + 1) * BW]
                    if g < 2:
                        nc.tensor.matmul(
                            po, AUG[0:2, g * H + jc * 128 : g * H + (jc + 1) * 128],
                            xs, start=True, stop=False,
                        )
                    else:
                        nc.tensor.matmul(
                            po, AUGNB[0:1, jc * 128 : (jc + 1) * 128],
                            os, start=True, stop=False,
                        )
                    for half in range(2):
                        bb = c * 2 + half
                        nc.tensor.matmul(
                            po,
                            wT8[:, bb * 256 : (bb + 1) * 256].rearrange(
                                "p (i j) -> p i j", i=2
                            ),
                            drhs(w, half),
                            start=False, stop=(half == 1),
                            perf_mode=DR,
                        )

        def emit_sig(w, t, ps, st):
            psr, psz, psn, psn2 = ps
            rz = wpool.tile([128, 8 * BW], BF16, tag=f"rz{w}", name=f"rz{w}_{t}")
            nc.scalar.activation(rz[:, 0 : 4 * BW], psr[:, :], AF.Sigmoid,
                                 scale=1.0 / WSC)
            nc.scalar.activation(rz[:, 4 * BW : 8 * BW], psz[:, :], AF.Sigmoid,
                                 scale=1.0 / WSC)
            st["rz"] = rz

        def emit_pool(w, t, st):
            # off-critical-path: q = z*h_old on GPSIMD
            rz = st["rz"]
            q = wpool.tile([128, 4 * BW], BF16, tag=f"q{w}", name=f"q{w}_{t}")
            nc.gpsimd.tensor_tensor(q[:, :], rz[:, 4 * BW : 8 * BW], hT8[w][:, :],
                                    ALU.mult)
            st["q"] = q

        def emit_m(w, t, ps, st):
            psr, psz, psn, psn2 = ps
            rz = st["rz"]
            m = wpool.tile([128, 4 * BW], BF16, tag=f"m{w}", name=f"m{w}_{t}")
            nc.vector.scalar_tensor_tensor(
                m[:, :], psn[:, :], 1.0 / WSC, rz[:, 0 : 4 * BW],
                ALU.mult, ALU.mult,
            )
            cv = wpool.tile([128, 4 * BW], BF16, tag=f"cv{w}", name=f"cv{w}_{t}")
            nc.vector.tensor_scalar(cv[:, :], rz[:, 4 * BW : 8 * BW], 1.0, -1.0,
                                    ALU.subtract, ALU.mult)
            st["m"], st["cv"] = m, cv

        def emit_gpairs(w, t, ps, st):
            # psn2 = gx_n + m via PE: aug pairs with identity-matmul accumulate
            psr, psz, psn, psn2 = ps
            m = st["m"]
            xs = xaug[0:2, t * B + w * BW : t * B + (w + 1) * BW]
            for jc in range(NJ):
                nc.tensor.matmul(
                    psn2[:, jc * BW : (jc + 1) * BW],
                    AUGG[0:2, jc * 128 : (jc + 1) * 128],
                    xs, start=True, stop=False,
                )
                nc.tensor.matmul(
                    psn2[:, jc * BW : (jc + 1) * BW],
                    identb[:, :],
                    m[:, jc * BW : (jc + 1) * BW],
                    start=False, stop=True,
                )

        def emit_tanh(w, t, ps, st):
            psr, psz, psn, psn2 = ps
            n = wpool.tile([128, 4 * BW], BF16, tag=f"n{w}", name=f"n{w}_{t}")
            nc.scalar.activation(n[:, :], psn2[:, :], AF.Tanh)
            st["n"] = n

        def emit_tail(w, t, st):
            n, cv, q = st["n"], st["cv"], st["q"]
            u = wpool.tile([128, 4 * BW], BF16, tag=f"u{w}", name=f"u{w}_{t}")
            nc.vector.tensor_tensor(u[:, :], cv[:, :], n[:, :], ALU.mult)
            hn = spool.tile([128, NK * BW], FP8, tag=f"hT{w}", name=f"hT{w}_{t}")
            nc.vector.tensor_tensor(hn[:, :], u[:, :], q[:, :], ALU.add)
            hT8[w] = hn

        # ---- the recurrence, fully unrolled, 2 waves interleaved ----
        sts = [{}, {}]
        pss = [None, None]
        for t in range(T):
            for w in range(W):
                psr = ppool.tile([128, 4 * BW], F32, tag=f"psR{w}", bufs=1,
                                 name=f"psr{w}_{t}")
                psz = ppool.tile([128, 4 * BW], F32, tag=f"psZ{w}", bufs=1,
                                 name=f"psz{w}_{t}")
                psn = ppool.tile([128, 4 * BW], F32, tag=f"psN{w}", bufs=1,
                                 name=f"psn{w}_{t}")
                psn2 = ppool.tile([128, 4 * BW], F32, tag="psPREP", bufs=2,
                                  name=f"psn2_{w}_{t}")
                pss[w] = (psr, psz, psn, psn2)
                if w == 0:
                    emit_pe(0, t, pss[0])
                    emit_sig(0, t, pss[0], sts[0])
                else:
                    emit_pe(1, t, pss[1])
                    emit_m(0, t, pss[0], sts[0])
                    emit_pool(0, t, sts[0])
                    emit_sig(1, t, pss[1], sts[1])
                    emit_gpairs(0, t, pss[0], sts[0])
                    emit_tanh(0, t, pss[0], sts[0])
                    emit_m(1, t, pss[1], sts[1])
                    emit_pool(1, t, sts[1])
                    emit_tail(0, t, sts[0])
                    emit_gpairs(1, t, pss[1], sts[1])
                    emit_tanh(1, t, pss[1], sts[1])
                    emit_tail(1, t, sts[1])

        # ---- head: out = relu(h) @ fc_w.T + fc_b ----
        pso = ppool.tile([B, 1], F32, tag="psPREP", bufs=2, name="ps_fc")
        for w in range(W):
            reluh = wpool.tile([128, NK * BW], F32, tag=f"relu{w}", name=f"relu{w}")
            nc.scalar.activation(reluh[:, :], hT8[w][:, :], AF.Relu)
            po = pso[w * BW : (w + 1) * BW, :]
            nc.tensor.matmul(
                po, onesf[:, 0:BW], fcbf[0:1, 0:1], start=True, stop=False
            )
            for k in range(NK):
                nc.tensor.matmul(
                    po,
                    reluh[:, k * BW : (k + 1) * BW],
                    fcwf[:, k : k + 1],
                    start=False, stop=(k == NK - 1),
                )
        outw = wpool.tile([B, 1], F32, tag="outw", name="out_sb")
        nc.vector.tensor_copy(outw[:, :], pso[:, :])
        nc.sync.dma_start(out=out_d[:, :], in_=outw[:, :])


_NC_CACHE: dict[int, bass.Bass] = {}


def _get_nc(T: int = T_FULL) -> bass.Bass:
    if T not in _NC_CACHE:
        _NC_CACHE[T] = build_nc(T)
    return _NC_CACHE[T]


def kernel(x, w_ih, w_hh, b_ih, b_hh, fc_w, fc_b, _trace=False, _tmpdir=None):
    x = np.ascontiguousarray(np.asarray(x, dtype=np.float32))
    nc = _get_nc(x.shape[1])
    shared = {
        "w_hh": np.ascontiguousarray(np.asarray(w_hh, np.float32)),
        "w_ih": np.ascontiguousarray(np.asarray(w_ih, np.float32)),
        "b_ih": np.ascontiguousarray(np.asarray(b_ih, np.float32)),
        "b_hh": np.ascontiguousarray(np.asarray(b_hh, np.float32)),
        "fc_w": np.ascontiguousarray(np.asarray(fc_w, np.float32)),
        "fc_b": np.ascontiguousarray(np.asarray(fc_b, np.float32)),
    }
    in_maps = [{"x": x[c * B : (c + 1) * B], **shared} for c in range(N_CORES)]
    res = run_bass_kernel_spmd(
        nc, in_maps, list(range(N_CORES)), trace=_trace, tmpdir=_tmpdir
    )
    out = np.concatenate([res.results[c]["out"] for c in range(N_CORES)], axis=0)
    if _trace:
        return out, res
    return out
